# revision 1
# baseline (speedup 1.0000x reference)
"""Self-contained Trainium2 Bass kernel for nn_GCMCModel (GCMC GNN).
Accepts FULL inputs, shards across 8 NeuronCores internally, returns FULL output.
"""

# ---- toolchain workarounds (this container's walrus supports only one
# sync-wait per instruction) -------------------------------------------------

def _apply_tile_fix():
    import concourse.mybir as mybir
    from concourse.tile import TileContext, ScopedClock
    if getattr(TileContext, "_drain_patched", False):
        return
    TileContext._drain_patched = True

    def _drain_and_barrier(self, tick_clock, wait_clock):
        nop = self.nc.sync.nop()
        wait_clock.add_sem_waits(nop.ins, ScopedClock({None: tick_clock.global_clock}))
        si = nop.ins.sync_info
        waits = list(si.on_wait) if si is not None else []
        if waits:
            si.on_wait = waits[:1]
        for w in waits[1:]:
            n2 = self.nc.sync.nop()
            n2.ins.sync_info = mybir.SyncInfo(on_wait=[w], on_update=[])
        self.nc.sync.drain()
        self.nc.all_engine_barrier()
        popped = self.nc._tile_sem_poison_stack.pop()
        assert popped is self._sem_poison
        self.nc.clear_and_free_semaphores(list(self.sems.allocated().values()))
        self.nc.all_engine_barrier()

    TileContext._drain_and_barrier = _drain_and_barrier


def _apply_bir_fix():
    import json as _json
    import concourse.bass_utils as _bu
    import concourse.bass2jax as _b2j
    if getattr(_bu, "_wait_split_patched", False):
        return
    _bu._wait_split_patched = True
    _orig = _bu.compile_bir_kernel
    _ctr = [0]

    def _split(bir_bytes):
        mod = _json.loads(bir_bytes)
        changed = False
        for fn in mod.get("functions", []):
            for blk in fn.get("blocks", []) or []:
                out = []
                for ins in blk.get("instructions", []):
                    si = ins.get("sync_info")
                    waits = (si or {}).get("on_wait") or []
                    if len(waits) > 1:
                        changed = True
                        for w in waits[:-1]:
                            _ctr[0] += 1
                            out.append({"debug": ins.get("debug", 0),
                                        "engine": ins["engine"], "ins": [],
                                        "name": f"{ins['name']}-ws{_ctr[0]}",
                                        "opcode": "NoOp", "outs": [],
                                        "sync_info": {"on_update": [],
                                                      "on_wait": [w]}})
                        si["on_wait"] = [waits[-1]]
                    out.append(ins)
                blk["instructions"] = out
        return _json.dumps(mod).encode() if changed else bir_bytes

    def _patched(bir_json, tmpdir, neff_name="file.neff"):
        if isinstance(bir_json, str):
            bir_json = bir_json.encode()
        return _orig(_split(bir_json), tmpdir, neff_name)

    _bu.compile_bir_kernel = _patched
    _b2j.compile_bir_kernel = _patched

_apply_tile_fix()
_apply_bir_fix()

import time as _time
import numpy as np
import concourse.bacc as bacc
import concourse.mybir as mybir
from concourse.tile import TileContext
from concourse import bass_utils

EXEC_SECONDS = []

N_CORES = 8
P = 128
GG = 32          # tiles per dma_gather group (<= 4096 idxs)
UHALF = 65024    # user table split point (even, 508*128)


def _build_side(n_slots, slot_of_edge, val_idx, val_par, val_half, n_cores):
    """Bin edges by (core, window, half) into a STRUCTURALLY UNIFORM tile grid:
    every core gets Wc windows x (KA half-A tiles + KB half-B tiles). Tile t:
    window = t // K, half = 0 if t % K < KA else 1, acc offset = window*128.
    Pad slots use value-idx 0 (a zero row), so they contribute nothing."""
    w = (slot_of_edge >> 7).astype(np.int64)
    n_win = (n_slots + 127) // 128
    Wc = (n_win + n_cores - 1) // n_cores
    r = (slot_of_edge & 127).astype(np.int64)
    halves = val_half if val_half is not None else np.zeros(len(w), np.int8)

    key = w * 2 + halves
    order = np.argsort(key, kind="stable")
    key_s = key[order]
    starts = np.searchsorted(key_s, np.arange(n_win * 2))
    ends = np.searchsorted(key_s, np.arange(n_win * 2) + 1)
    cnt = (ends - starts).reshape(n_win, 2)
    KA = max(1, int(np.ceil(cnt[:, 0].max() / P))) if cnt[:, 0].max() else 1
    KB = int(np.ceil(cnt[:, 1].max() / P)) if val_half is not None and cnt[:, 1].max() else 0
    K = KA + KB
    T = Wc * K

    half_tile = np.zeros(T, np.int8)
    off_tile = np.zeros(T, np.int64)
    for t in range(T):
        off_tile[t] = (t // K) * 128
        half_tile[t] = 0 if (t % K) < KA else 1

    per_core = []
    for c in range(n_cores):
        r_grid = np.zeros((P, T), np.float16)
        p_grid = np.zeros((P, T), np.uint8)
        idx_grid = np.zeros((T, P), np.int16)
        for li in range(Wc):
            win = li * n_cores + c
            if win >= n_win:
                continue
            for h, base_t, cap in ((0, li * K, KA), (1, li * K + KA, KB)):
                g0, g1 = starts[win * 2 + h], ends[win * 2 + h]
                eids = order[g0:g1]
                assert len(eids) <= cap * P
                for j, s0 in enumerate(range(0, len(eids), P)):
                    seg = eids[s0:s0 + P]
                    n = len(seg)
                    t = base_t + j
                    r_grid[:n, t] = r[seg]
                    p_grid[:n, t] = val_par[seg]
                    idx_grid[t, :n] = val_idx[seg]
        per_core.append(dict(r_grid=r_grid, p_grid=p_grid, idx_grid=idx_grid,
                             half_tile=half_tile, off_tile=off_tile))
    return per_core, Wc, T


def _wrap_idx(idx_flat):
    """[N] int16 -> [128, N/16] wrapped (16-lane wrap, replicated x8)."""
    n = len(idx_flat)
    assert n % 16 == 0
    w = idx_flat.reshape(n // 16, 16).T
    return np.tile(w, (8, 1)).copy()


def build_launch1(T_u, off_u, half_u, T_i, off_i, half_i, n_item_rows, n_ua, n_ub):
    """Aggregation kernel. half/off lists are identical across cores."""
    nc = bacc.Bacc()
    dt = mybir.dt
    itab = nc.dram_tensor("itab", [n_item_rows, 128], dt.float32, kind="ExternalInput")
    utabA = nc.dram_tensor("utabA", [n_ua, 128], dt.float32, kind="ExternalInput")
    utabB = nc.dram_tensor("utabB", [n_ub, 128], dt.float32, kind="ExternalInput")
    rg_u = nc.dram_tensor("rg_u", [P, T_u], dt.float16, kind="ExternalInput")
    pg_u = nc.dram_tensor("pg_u", [P, T_u], dt.uint8, kind="ExternalInput")
    ix_u = nc.dram_tensor("ix_u", [P, T_u * 8], dt.int16, kind="ExternalInput")
    rg_i = nc.dram_tensor("rg_i", [P, T_i], dt.float16, kind="ExternalInput")
    pg_i = nc.dram_tensor("pg_i", [P, T_i], dt.uint8, kind="ExternalInput")
    ix_i = nc.dram_tensor("ix_i", [P, T_i * 8], dt.int16, kind="ExternalInput")
    iota = nc.dram_tensor("iota", [128, 128], dt.float16, kind="ExternalInput")
    iota32 = nc.dram_tensor("iota32", [128, 128], dt.float32, kind="ExternalInput")
    nrg_u = nc.dram_tensor("nrg_u", [P, T_u], dt.float32, kind="ExternalInput")
    nrg_i = nc.dram_tensor("nrg_i", [P, T_i], dt.float32, kind="ExternalInput")
    W_u = (max(off_u) // 128) + 1 if len(off_u) else 1
    W_i = (max(off_i) // 128) + 1 if len(off_i) else 1
    acc_u_d = nc.dram_tensor("acc_u", [64, W_u * 128], dt.float32, kind="ExternalOutput")
    acc_i_d = nc.dram_tensor("acc_i", [64, W_i * 128], dt.float32, kind="ExternalOutput")

    with TileContext(nc) as tc:
        with tc.tile_pool(name="g", bufs=4) as gp, \
             tc.tile_pool(name="w", bufs=8) as wp, \
             tc.tile_pool(name="ps", bufs=8, space="PSUM") as pp, \
             tc.tile_pool(name="st", bufs=1) as st:
            iota_t = st.tile([128, 128], dt.float16)
            nc.sync.dma_start(out=iota_t[:], in_=iota[:, :])
            iota32_t = st.tile([128, 128], dt.float32)
            nc.sync.dma_start(out=iota32_t[:], in_=iota32[:, :])
            ones_t = st.tile([128, 1], dt.float32)
            nc.vector.memset(ones_t[:], 1.0)
            nrg_ut = st.tile([P, T_u], dt.float32)
            nrg_it = st.tile([P, T_i], dt.float32)
            nc.sync.dma_start(out=nrg_ut[:], in_=nrg_u[:, :])
            nc.sync.dma_start(out=nrg_it[:], in_=nrg_i[:, :])
            acc_u = st.tile([64, W_u * 128], dt.float32)
            acc_i = st.tile([64, W_i * 128], dt.float32)
            nc.vector.memset(acc_u[:], 0.0)
            nc.vector.memset(acc_i[:], 0.0)
            rg_ut = st.tile([P, T_u], dt.float16)
            pg_ut = st.tile([P, T_u], dt.uint8)
            nc.sync.dma_start(out=rg_ut[:], in_=rg_u[:, :])
            nc.sync.dma_start(out=pg_ut[:], in_=pg_u[:, :])
            rg_it = st.tile([P, T_i], dt.float16)
            pg_it = st.tile([P, T_i], dt.uint8)
            nc.sync.dma_start(out=rg_it[:], in_=rg_i[:, :])
            nc.sync.dma_start(out=pg_it[:], in_=pg_i[:, :])
            ix_ut = st.tile([P, T_u * 8], dt.int16)
            ix_it = st.tile([P, T_i * 8], dt.int16)
            nc.sync.dma_start(out=ix_ut[:], in_=ix_u[:, :])
            nc.sync.dma_start(out=ix_it[:], in_=ix_i[:, :])

            for side in ("u", "i"):
                T = T_u if side == "u" else T_i
                offs = off_u if side == "u" else off_i
                halves = half_u if side == "u" else half_i
                rg = rg_ut if side == "u" else rg_it
                nrg = nrg_ut if side == "u" else nrg_it
                pg = pg_ut if side == "u" else pg_it
                ix = ix_ut if side == "u" else ix_it
                acc = acc_u if side == "u" else acc_i
                # gather groups: runs of tiles sharing a table
                groups = []
                t0 = 0
                while t0 < T:
                    t1 = t0
                    while t1 < T and t1 - t0 < GG and halves[t1] == halves[t0]:
                        t1 += 1
                    groups.append((t0, t1))
                    t0 = t1
                # issue gathers; remember which vp tile holds each tile's rows
                vp_of = {}
                for (a, b) in groups:
                    nt = b - a
                    if side == "u":
                        tab = itab
                    else:
                        tab = utabA if halves[a] == 0 else utabB
                    vp = gp.tile([P, GG, 128], dt.float32, tag="vp")
                    nc.gpsimd.dma_gather(
                        out_ap=vp[:, :nt, :], in_ap=tab[:, :],
                        idxs_ap=ix[:, a * 8:b * 8],
                        num_idxs=nt * 128, num_idxs_reg=nt * 128,
                        elem_size=128, single_packet=False)
                    for t in range(a, b):
                        vp_of[t] = (vp, t - a)
                # per window: accumulate K tiles in PSUM, one acc add at the end
                K = T // ((max(offs) // 128) + 1)
                t = 0
                while t < T:
                    o = int(offs[t])
                    K_w = 1
                    while t + K_w < T and int(offs[t + K_w]) == o:
                        K_w += 1
                    ps = pp.tile([64, 128], dt.float32, tag="ps")
                    for j in range(K_w):
                        tt = t + j
                        vp, vi = vp_of[tt]
                        oh = wp.tile([P, 128], dt.float32, tag="oh")
                        if tt % 2 == 0:
                            nc.vector.tensor_tensor(
                                out=oh[:], in0=rg[:, tt:tt + 1].to_broadcast([P, 128]),
                                in1=iota_t[:],
                                op=mybir.AluOpType.is_equal)
                        else:
                            ab = wp.tile([P, 128], dt.float32, tag="ab")
                            nc.scalar.activation(
                                ab[:], iota32_t[:], mybir.ActivationFunctionType.Abs,
                                bias=nrg[:, tt:tt + 1], scale=1.0)
                            nc.scalar.activation(
                                oh[:], ab[:], mybir.ActivationFunctionType.Relu,
                                bias=ones_t[:], scale=-1.0)
                        vsel = wp.tile([P, 64], dt.float32, tag="vsel")
                        nc.vector.select(
                            out=vsel[:],
                            mask=pg[:, tt:tt + 1].to_broadcast([P, 64]),
                            on_true=vp[:, vi, 64:128],
                            on_false=vp[:, vi, 0:64])
                        nc.tensor.matmul(ps[:], lhsT=vsel[:], rhs=oh[:],
                                         start=(j == 0), stop=(j == K_w - 1))
                    nc.vector.tensor_add(
                        out=acc[:, o:o + 128], in0=acc[:, o:o + 128], in1=ps[:])
                    t += K_w
            nc.sync.dma_start(out=acc_u_d[:, :], in_=acc_u[:])
            nc.sync.dma_start(out=acc_i_d[:, :], in_=acc_i[:])
    nc.compile()
    return nc


def build_launch2(Bc):
    """GCN + MLP for Bc outputs per core, [feature, batch] layout."""
    nc = bacc.Bacc()
    dt = mybir.dt
    ue = nc.dram_tensor("ue", [64, Bc], dt.float32, kind="ExternalInput")
    ie = nc.dram_tensor("ie", [64, Bc], dt.float32, kind="ExternalInput")
    gu = nc.dram_tensor("gu", [64, Bc], dt.float32, kind="ExternalInput")  # agg_u/deg_i at item_id (gcn_user_h^T)
    gi = nc.dram_tensor("gi", [64, Bc], dt.float32, kind="ExternalInput")  # agg_i/deg_u at user_id (gcn_item_h^T)
    Wu = nc.dram_tensor("Wu", [64, 64], dt.float32, kind="ExternalInput")
    Wi = nc.dram_tensor("Wi", [64, 64], dt.float32, kind="ExternalInput")
    bu = nc.dram_tensor("bu", [64, 1], dt.float32, kind="ExternalInput")
    bi = nc.dram_tensor("bi", [64, 1], dt.float32, kind="ExternalInput")
    W1 = nc.dram_tensor("W1", [256, 128], dt.float32, kind="ExternalInput")
    b1 = nc.dram_tensor("b1", [128, 1], dt.float32, kind="ExternalInput")
    W2 = nc.dram_tensor("W2", [128, 64], dt.float32, kind="ExternalInput")
    b2 = nc.dram_tensor("b2", [64, 1], dt.float32, kind="ExternalInput")
    W3 = nc.dram_tensor("W3", [64, 1], dt.float32, kind="ExternalInput")
    bias = nc.dram_tensor("bias", [1, Bc], dt.float32, kind="ExternalInput")  # b3+ub+ib
    out = nc.dram_tensor("out", [1, Bc], dt.float32, kind="ExternalOutput")
    CH = 512
    with TileContext(nc) as tc:
        with tc.tile_pool(name="p", bufs=1) as pool, \
             tc.tile_pool(name="ps", bufs=1, space="PSUM") as pp:
            t_ue = pool.tile([64, Bc], dt.float32)
            t_ie = pool.tile([64, Bc], dt.float32)
            t_gu = pool.tile([64, Bc], dt.float32)
            t_gi = pool.tile([64, Bc], dt.float32)
            for t, d in ((t_ue, ue), (t_ie, ie), (t_gu, gu), (t_gi, gi)):
                nc.sync.dma_start(out=t[:], in_=d[:, :])
            t_Wu = pool.tile([64, 64], dt.float32)
            t_Wi = pool.tile([64, 64], dt.float32)
            t_W2 = pool.tile([128, 64], dt.float32)
            t_W3 = pool.tile([64, 1], dt.float32)
            t_W1 = pool.tile([64, 4 * 128], dt.float32)
            for t, d in ((t_Wu, Wu), (t_Wi, Wi), (t_W2, W2), (t_W3, W3)):
                nc.sync.dma_start(out=t[:], in_=d[:, :])
            for k in range(4):
                nc.sync.dma_start(out=t_W1[:, 128 * k:128 * k + 128],
                                  in_=W1[64 * k:64 * k + 64, :])
            t_bu = pool.tile([64, 1], dt.float32)
            t_bi = pool.tile([64, 1], dt.float32)
            t_b1 = pool.tile([128, 1], dt.float32)
            t_b2 = pool.tile([64, 1], dt.float32)
            for t, d in ((t_bu, bu), (t_bi, bi), (t_b1, b1), (t_b2, b2)):
                nc.sync.dma_start(out=t[:], in_=d[:, :])
            t_bias = pool.tile([1, Bc], dt.float32)
            nc.sync.dma_start(out=t_bias[:], in_=bias[:, :])

            guo = pool.tile([64, Bc], dt.float32)
            gio = pool.tile([64, Bc], dt.float32)
            h1 = pool.tile([128, Bc], dt.float32)
            h2 = pool.tile([64, Bc], dt.float32)
            res = pool.tile([1, Bc], dt.float32)
            for c0 in range(0, Bc, CH):
                c1 = min(c0 + CH, Bc)
                # gcn outs: relu(W^T @ g + b)
                p1 = pp.tile([64, CH], dt.float32, tag="p1")
                nc.tensor.matmul(p1[:, :c1 - c0], lhsT=t_Wu[:], rhs=t_gu[:, c0:c1],
                                 start=True, stop=True)
                nc.scalar.activation(guo[:, c0:c1], p1[:, :c1 - c0],
                                     mybir.ActivationFunctionType.Relu,
                                     bias=t_bu[:], scale=1.0)
                p2 = pp.tile([64, CH], dt.float32, tag="p2")
                nc.tensor.matmul(p2[:, :c1 - c0], lhsT=t_Wi[:], rhs=t_gi[:, c0:c1],
                                 start=True, stop=True)
                nc.scalar.activation(gio[:, c0:c1], p2[:, :c1 - c0],
                                     mybir.ActivationFunctionType.Relu,
                                     bias=t_bi[:], scale=1.0)
                # products
                prods = []
                for (x_, y_) in ((t_ue, t_ie), (t_ue, gio), (guo, t_ie), (guo, gio)):
                    pr = pool.tile([64, CH], dt.float32, tag=f"pr{len(prods)}")
                    nc.vector.tensor_mul(pr[:, :c1 - c0], x_[:, c0:c1], y_[:, c0:c1])
                    prods.append(pr)
                # x @ W1 (+b1) tanh : accumulate 4 chunks
                p3 = pp.tile([128, CH], dt.float32, tag="p3")
                for k in range(4):
                    nc.tensor.matmul(p3[:, :c1 - c0], lhsT=t_W1[:, 128 * k:128 * k + 128],
                                     rhs=prods[k][:, :c1 - c0],
                                     start=(k == 0), stop=(k == 3))
                nc.scalar.activation(h1[:, c0:c1], p3[:, :c1 - c0],
                                     mybir.ActivationFunctionType.Tanh,
                                     bias=t_b1[:], scale=1.0)
                p4 = pp.tile([64, CH], dt.float32, tag="p4")
                nc.tensor.matmul(p4[:, :c1 - c0], lhsT=t_W2[:], rhs=h1[:, c0:c1],
                                 start=True, stop=True)
                nc.scalar.activation(h2[:, c0:c1], p4[:, :c1 - c0],
                                     mybir.ActivationFunctionType.Tanh,
                                     bias=t_b2[:], scale=1.0)
                p5 = pp.tile([1, CH], dt.float32, tag="p5")
                nc.tensor.matmul(p5[:, :c1 - c0], lhsT=t_W3[:], rhs=h2[:, c0:c1],
                                 start=True, stop=True)
                nc.vector.tensor_add(res[:, c0:c1], p5[:, :c1 - c0], t_bias[:, c0:c1])
            nc.sync.dma_start(out=out[:, :], in_=res[:])
    nc.compile()
    return nc


def kernel(user_table, item_table, Wu, bu, Wi, bi, W1, b1, W2, b2, W3, b3,
           user_bias, item_bias, user_id, item_id, edge_user, edge_item):
    EXEC_SECONDS.clear()
    user_table = np.asarray(user_table, np.float32)
    item_table = np.asarray(item_table, np.float32)
    user_id = np.asarray(user_id).astype(np.int64)
    item_id = np.asarray(item_id).astype(np.int64)
    eu = np.asarray(edge_user).astype(np.int64)
    ei = np.asarray(edge_item).astype(np.int64)
    N_USER, D = user_table.shape
    N_ITEM = item_table.shape[0]
    B = len(user_id)
    E = len(eu)

    # ---- host prep ----
    uu = np.unique(user_id)
    ui = np.unique(item_id)
    pos_u = np.full(N_USER, -1, np.int64); pos_u[uu] = np.arange(len(uu))
    pos_i = np.full(N_ITEM, -1, np.int64); pos_i[ui] = np.arange(len(ui))

    deg_u_full = np.bincount(eu, minlength=N_USER).astype(np.float32) + 1.0
    deg_i_full = np.bincount(ei, minlength=N_ITEM).astype(np.float32) + 1.0

    # user-side: slots over users, values = item pair-rows
    su = pos_u[eu]
    mu = su >= 0
    vi_u = ((ei[mu] >> 1) + 1).astype(np.int16)
    vp_u = (ei[mu] & 1).astype(np.float16)
    side_u, Wc_u, T_u = _build_side(len(uu), su[mu], vi_u, vp_u, None, N_CORES)

    # item-side: slots over items, values = user pair-rows (two halves)
    si = pos_i[ei]
    mi = si >= 0
    uh = (eu[mi] >= UHALF).astype(np.int8)
    rel = eu[mi] - uh.astype(np.int64) * UHALF
    vi_i = ((rel >> 1) + 1).astype(np.int16)
    vp_i = (rel & 1).astype(np.float16)
    side_i, Wc_i, T_i = _build_side(len(ui), si[mi], vi_i, vp_i, uh, N_CORES)

    # value tables with leading zero pair-row
    def pairs(tb):
        n = tb.shape[0]
        pad = (-n) % 2
        tbp = np.vstack([np.zeros((2, 64), np.float32), tb,
                         np.zeros((pad, 64), np.float32)])
        return np.ascontiguousarray(tbp.reshape(-1, 128))
    itab = pairs(item_table)
    utabA = pairs(user_table[:UHALF])
    utabB = pairs(user_table[UHALF:])

    # tile metadata must be identical across cores for SPMD: they are, by
    # construction (off/half derive from the same Wc/K/t ordering) — verify.
    off_u0 = side_u[0]["off_tile"]; half_u0 = side_u[0]["half_tile"]
    off_i0 = side_i[0]["off_tile"]; half_i0 = side_i[0]["half_tile"]
    for c in range(1, N_CORES):
        assert (side_u[c]["off_tile"] == off_u0).all()
        assert (side_i[c]["off_tile"] == off_i0).all()
        assert (side_u[c]["half_tile"] == half_u0).all()
        assert (side_i[c]["half_tile"] == half_i0).all()

    nc1 = build_launch1(T_u, off_u0, half_u0, T_i, off_i0, half_i0,
                        itab.shape[0], utabA.shape[0], utabB.shape[0])
    iota = np.broadcast_to(np.arange(128, dtype=np.float16), (128, 128)).copy()
    in_maps = []
    for c in range(N_CORES):
        du, di = side_u[c], side_i[c]
        in_maps.append(dict(
            itab=itab, utabA=utabA, utabB=utabB, iota=iota,
            iota32=np.broadcast_to(np.arange(128, dtype=np.float32),
                                   (128, 128)).copy(),
            nrg_u=-du["r_grid"].astype(np.float32),
            nrg_i=-di["r_grid"].astype(np.float32),
            rg_u=du["r_grid"], pg_u=du["p_grid"],
            ix_u=_wrap_idx(du["idx_grid"].reshape(-1)),
            rg_i=di["r_grid"], pg_i=di["p_grid"],
            ix_i=_wrap_idx(di["idx_grid"].reshape(-1)),
        ))
    _t0 = _time.perf_counter()
    res1 = bass_utils.run_bass_kernel_spmd(nc1, in_maps, core_ids=list(range(N_CORES)))
    EXEC_SECONDS.append(_time.perf_counter() - _t0)

    # reassemble aggs: slot s lives on core (s>>7)%8 at local window ((s>>7)//8)
    def unpack(key, n_slots):
        agg = np.zeros((n_slots, 64), np.float32)
        s = np.arange(n_slots)
        w = s >> 7
        core = w % N_CORES
        loc = (w // N_CORES) * 128 + (s & 127)
        for c in range(N_CORES):
            m = core == c
            agg[m] = res1.results[c][key][:, loc[m]].T
        return agg
    agg_u_slots = unpack("acc_u", len(uu))   # per unique user: sum of item rows
    agg_i_slots = unpack("acc_i", len(ui))   # per unique item: sum of user rows

    # per-b features
    gcn_item_h = agg_u_slots[pos_u[user_id]] / deg_u_full[user_id][:, None]
    gcn_user_h = agg_i_slots[pos_i[item_id]] / deg_i_full[item_id][:, None]
    u_emb = user_table[user_id]
    i_emb = item_table[item_id]
    bias_b = (np.float32(b3[0]) + np.asarray(user_bias)[user_id, 0]
              + np.asarray(item_bias)[item_id, 0]).astype(np.float32)

    Bc = B // N_CORES
    nc2 = build_launch2(Bc)
    in2 = []
    for c in range(N_CORES):
        sl = slice(c * Bc, (c + 1) * Bc)
        in2.append(dict(
            ue=np.ascontiguousarray(u_emb[sl].T), ie=np.ascontiguousarray(i_emb[sl].T),
            gu=np.ascontiguousarray(gcn_user_h[sl].T),
            gi=np.ascontiguousarray(gcn_item_h[sl].T),
            Wu=np.asarray(Wu, np.float32), Wi=np.asarray(Wi, np.float32),
            bu=np.asarray(bu, np.float32).reshape(64, 1),
            bi=np.asarray(bi, np.float32).reshape(64, 1),
            W1=np.asarray(W1, np.float32), b1=np.asarray(b1, np.float32).reshape(128, 1),
            W2=np.asarray(W2, np.float32), b2=np.asarray(b2, np.float32).reshape(64, 1),
            W3=np.asarray(W3, np.float32),
            bias=bias_b[sl].reshape(1, Bc),
        ))
    _t0 = _time.perf_counter()
    res2 = bass_utils.run_bass_kernel_spmd(nc2, in2, core_ids=list(range(N_CORES)))
    EXEC_SECONDS.append(_time.perf_counter() - _t0)
    out = np.concatenate([res2.results[c]["out"][0] for c in range(N_CORES)])
    return out.astype(np.float32)

